# revision 1
# baseline (speedup 1.0000x reference)
"""Trainium2 Bass kernel for nn_Detector (GNN message passing).

Math: the reference's per-iteration edge aggregation
    agg = segment_sum((h[src] + ef_w[ef]) * valid, by=ed)[:N] / cnt
is linear in h and ef_w, so it factors through two tiny count histograms
built in ONE pass over the edge index arrays:
    C[d, s] = #valid edges s->d          (32x32)
    F[d, t] = #valid edges into d with feature t   (32x6)
    agg = (C @ h + F @ ef_w) / cnt,   cnt = max(rowsum(C), 1)
Out-of-range (padded) edges produce all-zero one-hot rows and drop out
automatically, matching the reference's valid-mask semantics.

Distribution: edges are sharded across 8 cores; each core builds partial
C|F [32,38] via one-hot matmuls (contraction over 128-edge chunks on the
PE), partials are AllGather'ed and summed, then every core runs the
identical 5-iteration GRU + head on [32,128] tiles; core 0's scalar is
returned.
"""

import ml_dtypes
import numpy as np

import concourse.bass as bass
import concourse.mybir as mybir
import concourse.tile as tile
from concourse.bass_utils import run_bass_kernel_spmd

dt = mybir.dt
AF = mybir.ActivationFunctionType
ALU = mybir.AluOpType

NCORES = 8
E_FULL = 400000
W = 392                    # edge columns per partition row
EPC = 128 * W              # 50176 padded edges per core
E_PAD = NCORES * EPC       # 401408
NGRP = W // 4              # 98 matmul groups of 4 chunks (512 edges)
DIM = 128
N = 32
EPS = 1e-5
RSQRT_MAGIC = 0x5F3759DF   # rsqrt bit-hack seed
MAX_WAITS = 1              # this walrus rejects >1 sync wait per instruction


def _split_excess_waits(nc):
    """Split instructions carrying more than MAX_WAITS sync-wait conditions
    into preceding same-engine NOPs (walrus codegen limit)."""
    for blk in nc.main_func.blocks:
        insts = blk.instructions
        i = 0
        while i < len(insts):
            inst = insts[i]
            si = inst.sync_info
            if si is not None and len(si.on_wait) > MAX_WAITS:
                waits = list(si.on_wait)
                keep = waits[-MAX_WAITS:]
                rest = waits[:-MAX_WAITS]
                new_nops = []
                while rest:
                    chunk, rest = rest[:MAX_WAITS], rest[MAX_WAITS:]
                    nop = mybir.InstNoOp(
                        name=f"waitsplit-{nc.next_id()}", ins=[], outs=[])
                    nop.engine = inst.engine
                    nop.sync_info = mybir.SyncInfo(on_wait=chunk, on_update=[])
                    nc.register_instruction(nop, overwrite=True)
                    new_nops.append(nop)
                inst.sync_info = mybir.SyncInfo(
                    on_wait=keep, on_update=list(si.on_update))
                for j, nop in enumerate(new_nops):
                    insts.insert(i + j, nop)
                i += len(new_nops)
            i += 1

f32 = dt.float32
bf16 = dt.bfloat16
i16 = dt.int16
i32 = dt.int32


def _sqrt_newton(nc, vp, u, tag_prefix):
    """1/sqrt(u) for u [P,1] fp32 in SBUF via the rsqrt bit-hack seed +
    2 Newton iterations using only mult/add (this walrus cannot encode
    AP-scalar divide, and ACT Sqrt would cost a ~2.7us table switch).
    Returns ([P,1] inv_sigma AP, None)."""
    P = u.shape[0]
    y = vp.tile([P, 1], f32, name=f"{tag_prefix}_y")
    a = vp.tile([P, 1], f32, name=f"{tag_prefix}_a")
    # y0 bits = MAGIC - (u_bits >> 1), via c - x = (~x) + (c + 1)
    # (bitwise and arith ALU ops cannot share one instruction)
    nc.vector.tensor_scalar(
        y.bitcast(i32), u.bitcast(i32), 1, None, ALU.logical_shift_right)
    nc.vector.tensor_scalar(
        y.bitcast(i32), y.bitcast(i32), -1, None, ALU.bitwise_xor)
    nc.vector.tensor_scalar(
        y.bitcast(i32), y.bitcast(i32), RSQRT_MAGIC + 1, None, ALU.add)
    for _ in range(2):
        nc.vector.tensor_mul(a, y, y)                             # y^2
        nc.vector.tensor_mul(a, a, u)                             # u*y^2
        nc.vector.tensor_scalar(a, a, -0.5, 1.5, ALU.mult, ALU.add)
        nc.vector.tensor_mul(y, y, a)                             # Newton
    return y, None


def build_program():
    # this walrus snapshot cannot encode the Pool RANGE_CLEAR InstISA that
    # TileContext's exit emits via clear_and_free_semaphores; skip the
    # sem-clear ISA (keep dma_reset + bookkeeping).  The NEFF is executed
    # freshly per load, so end-of-kernel sem hygiene is not load-bearing
    # here (verified by back-to-back runs in test.py).
    _orig_clear = bass.Bass.clear_and_free_semaphores

    def _clear_no_isa(self, sems):
        if not sems:
            return
        sem_nums = [
            s.num if isinstance(s, bass.SemaphoreHandle) else s for s in sems
        ]
        from concourse.bass import compact_to_ranges
        for sem_range in compact_to_ranges(sem_nums):
            self.gpsimd.dma_reset(sem_range)
        self._state.prepend_free_semaphores(sem_nums)
        for poison_set in self._tile_sem_poison_stack:
            poison_set.update(sem_nums)

    bass.Bass.clear_and_free_semaphores = _clear_no_isa
    try:
        return _build_program_inner()
    finally:
        bass.Bass.clear_and_free_semaphores = _orig_clear


def _build_program_inner():
    nc = bass.Bass(trn_type="TRN2")

    # ---- DRAM I/O ---------------------------------------------------------
    es_d = nc.dram_tensor("es", [128, 4 * W], i16, kind="ExternalInput")
    ed_d = nc.dram_tensor("ed", [128, 4 * W], i16, kind="ExternalInput")
    ef_d = nc.dram_tensor("ef", [128, 4 * W], i16, kind="ExternalInput")
    nt_d = nc.dram_tensor("nt", [32, 4], i16, kind="ExternalInput")
    tr_d = nc.dram_tensor("tr", [32, 4], i16, kind="ExternalInput")
    ne_w_d = nc.dram_tensor("ne_w", [20, DIM], f32, kind="ExternalInput")
    te_w_d = nc.dram_tensor("te_w", [6, DIM], f32, kind="ExternalInput")
    ef_w_d = nc.dram_tensor("ef_w", [6, DIM], f32, kind="ExternalInput")
    w_ih_d = nc.dram_tensor("w_ih", [3 * DIM, DIM], f32, kind="ExternalInput")
    w_hh_d = nc.dram_tensor("w_hh", [3 * DIM, DIM], f32, kind="ExternalInput")
    b_ih_d = nc.dram_tensor("b_ih", [1, 3 * DIM], f32, kind="ExternalInput")
    b_hh_d = nc.dram_tensor("b_hh", [1, 3 * DIM], f32, kind="ExternalInput")
    ln_g_d = nc.dram_tensor("ln_g", [DIM, 1], f32, kind="ExternalInput")
    ln_b_d = nc.dram_tensor("ln_b", [DIM, 1], f32, kind="ExternalInput")
    fc1_w_d = nc.dram_tensor("fc1_w", [DIM, 2 * DIM], f32, kind="ExternalInput")
    fc1_b_d = nc.dram_tensor("fc1_b", [DIM, 1], f32, kind="ExternalInput")
    ln2_g_d = nc.dram_tensor("ln2_g", [DIM, 1], f32, kind="ExternalInput")
    ln2_b_d = nc.dram_tensor("ln2_b", [DIM, 1], f32, kind="ExternalInput")
    fc2_w_d = nc.dram_tensor("fc2_w", [1, DIM], f32, kind="ExternalInput")
    fc2_b_d = nc.dram_tensor("fc2_b", [1, 1], f32, kind="ExternalInput")
    ident_d = nc.dram_tensor("ident128", [128, 128], f32, kind="ExternalInput")
    ones_r_d = nc.dram_tensor("ones_row", [1, 128], f32, kind="ExternalInput")
    ones_c_d = nc.dram_tensor("ones_col", [128, 1], f32, kind="ExternalInput")
    iota_c_d = nc.dram_tensor("iota_col", [128, 1], f32, kind="ExternalInput")
    iota_m_d = nc.dram_tensor("iota_mat", [32, 32], f32, kind="ExternalInput")
    iota_b_d = nc.dram_tensor("iota_row_bf", [128, 32], dt.bfloat16,
                              kind="ExternalInput")
    out_d = nc.dram_tensor("out", [1, 1], f32, kind="ExternalOutput")

    # collective bounce buffers (internal DRAM)
    ag_in = nc.dram_tensor("ag_in", [32, 38], f32)
    ag_out = nc.dram_tensor("ag_out", [32 * NCORES, 38], f32, addr_space="Shared")

    with tile.TileContext(nc) as tc:
        with (
            tc.tile_pool(name="cst", bufs=1) as cp,      # persistent SBUF
            tc.tile_pool(name="var", bufs=2) as vp,      # loop temporaries
            tc.tile_pool(name="ps", bufs=1, space="PSUM") as pp,
        ):
            # ================= constants / weights into SBUF ==============
            ident = cp.tile([128, 128], f32, name="ident")
            nc.sync.dma_start(ident, ident_d[:, :])
            ones_row = cp.tile([1, 128], f32, name="ones_row_sb")
            nc.sync.dma_start(ones_row, ones_r_d[:, :])
            ones_col = cp.tile([128, 1], f32, name="ones_col_sb")
            nc.sync.dma_start(ones_col, ones_c_d[:, :])
            iota_col = cp.tile([128, 1], f32, name="iota_col_sb")
            nc.sync.dma_start(iota_col, iota_c_d[:, :])
            iota_mat = cp.tile([32, 32], f32, name="iota_mat_sb")
            nc.sync.dma_start(iota_mat, iota_m_d[:, :])
            iota_bf = cp.tile([128, 32], bf16, name="iota_bf_sb")
            nc.sync.dma_start(iota_bf, iota_b_d[:, :])

            ne_w = cp.tile([20, DIM], f32, name="ne_w_sb")
            nc.sync.dma_start(ne_w, ne_w_d[:, :])
            te_w = cp.tile([6, DIM], f32, name="te_w_sb")
            nc.sync.dma_start(te_w, te_w_d[:, :])
            ef_w = cp.tile([6, DIM], f32, name="ef_w_sb")
            nc.sync.dma_start(ef_w, ef_w_d[:, :])
            b_ih = cp.tile([1, 384], f32, name="b_ih_sb")
            nc.sync.dma_start(b_ih, b_ih_d[:, :])
            b_hh = cp.tile([1, 384], f32, name="b_hh_sb")
            nc.sync.dma_start(b_hh, b_hh_d[:, :])
            ln_g = cp.tile([128, 1], f32, name="ln_g_sb")
            nc.sync.dma_start(ln_g, ln_g_d[:, :])
            ln_b = cp.tile([128, 1], f32, name="ln_b_sb")
            nc.sync.dma_start(ln_b, ln_b_d[:, :])
            fc1_b = cp.tile([128, 1], f32, name="fc1_b_sb")
            nc.sync.dma_start(fc1_b, fc1_b_d[:, :])
            ln2_g = cp.tile([128, 1], f32, name="ln2_g_sb")
            nc.sync.dma_start(ln2_g, ln2_g_d[:, :])
            ln2_b = cp.tile([128, 1], f32, name="ln2_b_sb")
            nc.sync.dma_start(ln2_b, ln2_b_d[:, :])
            fc2_col = cp.tile([128, 1], f32, name="fc2_col")
            nc.sync.dma_start(fc2_col, fc2_w_d.rearrange("a d -> d a"))
            fc2_b = cp.tile([1, 1], f32, name="fc2_b_sb")
            nc.sync.dma_start(fc2_b, fc2_b_d[:, :])

            bsum = cp.tile([1, 384], f32, name="bsum")
            nc.vector.tensor_add(bsum, b_ih, b_hh)

            # GRU weights, transposed to [dim_in(K)=128, gate] layout
            w_ihT = cp.tile([128, 384], f32, name="w_ihT")
            w_hhT = cp.tile([128, 384], f32, name="w_hhT")
            fc1_w = cp.tile([128, 256], f32, name="fc1_w_sb")
            nc.sync.dma_start(fc1_w, fc1_w_d[:, :])
            fc1T_a = cp.tile([128, 128], f32, name="fc1T_a")
            fc1T_b = cp.tile([128, 128], f32, name="fc1T_b")
            for gsrc, gdst in ((w_ih_d, w_ihT), (w_hh_d, w_hhT)):
                for g in range(3):
                    wchunk = vp.tile([128, 128], f32, name="wchunk", tag="wchunk")
                    nc.sync.dma_start(wchunk, gsrc[128 * g:128 * (g + 1), :])
                    wT_ps = pp.tile([128, 128], f32, name="wT_ps", tag="psA")
                    nc.tensor.transpose(wT_ps, wchunk, ident)
                    nc.scalar.copy(gdst[:, 128 * g:128 * (g + 1)], wT_ps)
            for g, gdst in enumerate((fc1T_a, fc1T_b)):
                wT_ps = pp.tile([128, 128], f32, name="wT_ps2", tag="psA")
                nc.tensor.transpose(wT_ps, fc1_w[:, 128 * g:128 * (g + 1)], ident)
                nc.scalar.copy(gdst, wT_ps)

            # ================= edge phase: build one-hots + histogram =====
            raw = cp.tile([128, 3 * 4 * W], i16, name="raw")
            nc.sync.dma_start(raw[:, 0:4 * W], es_d[:, :])
            nc.sync.dma_start(raw[:, 4 * W:8 * W], ed_d[:, :])
            nc.sync.dma_start(raw[:, 8 * W:12 * W], ef_d[:, :])

            # compact int64-low-halves (stride 4 int16) -> unit-stride bf16
            sd = cp.tile([128, 3 * W], bf16, name="sd")
            raw_v = raw.rearrange("p (c w f) -> p c w f", c=3, f=4)
            nc.vector.tensor_copy(
                sd.rearrange("p (c w) -> p c w", c=3), raw_v[:, :, :, 0])

            # one-hot builds:
            #  ohd chunk-major [128, w*32+d] (contiguous 1-dim weight APs)
            #  ohs/ohf value-major (unit-stride 16-bit sweeps; used as
            #  2-free-dim moving operands)
            ohd = cp.tile([128, W * 32], bf16, name="ohd")
            nc.vector.tensor_tensor(
                ohd.rearrange("p (c v) -> p c v", v=32),
                sd[:, W:2 * W].unsqueeze(2).broadcast_to([128, W, 32]),
                iota_bf.unsqueeze(1).broadcast_to([128, W, 32]),
                ALU.is_equal)
            ohs = cp.tile([128, 32 * W], bf16, name="ohs")
            ohf = cp.tile([128, 6 * W], bf16, name="ohf")
            for v in range(32):
                nc.vector.tensor_scalar(
                    ohs[:, v * W:(v + 1) * W], sd[:, 0:W],
                    float(v), None, ALU.is_equal)
            for v in range(6):
                nc.vector.tensor_scalar(
                    ohf[:, v * W:(v + 1) * W], sd[:, 2 * W:3 * W],
                    float(v), None, ALU.is_equal)

            # histogram matmuls: psum[(j,d),(i,s)] += D_j^T S_i over groups
            hist = pp.tile([128, 128], f32, name="hist", tag="psA")
            histf = pp.tile([128, 24], f32, name="histf", tag="psHF")
            ohs_r = ohs.rearrange("p (v c) -> p c v", v=32)   # [128, 392, 32]
            ohf_r = ohf.rearrange("p (v c) -> p c v", v=6)    # [128, 392, 6]
            for g in range(NGRP):
                lhsT = ohd[:, 128 * g:128 * (g + 1)]          # ed one-hots
                rhs_s = ohs_r[:, 4 * g:4 * g + 4, :]          # es one-hots
                rhs_f = ohf_r[:, 4 * g:4 * g + 4, :]
                nc.tensor.matmul(hist, lhsT, rhs_s,
                                 start=(g == 0), stop=(g == NGRP - 1))
                nc.tensor.matmul(histf, lhsT, rhs_f,
                                 start=(g == 0), stop=(g == NGRP - 1))

            # extract + sum the 4 diagonal blocks -> partial C [32,32], F [32,6]
            hs = cp.tile([128, 152], f32, name="hs")
            nc.scalar.copy(hs[:, 0:128], hist)
            nc.scalar.copy(hs[:, 128:152], histf)
            tmpc = cp.tile([32, 96], f32, name="tmpc")
            tmpf = cp.tile([32, 18], f32, name="tmpf")
            for j in range(1, 4):
                nc.sync.dma_start(
                    tmpc[:, 32 * (j - 1):32 * j],
                    hs[32 * j:32 * (j + 1), 32 * j:32 * (j + 1)])
                nc.sync.dma_start(
                    tmpf[:, 6 * (j - 1):6 * j],
                    hs[32 * j:32 * (j + 1), 128 + 6 * j:128 + 6 * (j + 1)])
            pk = cp.tile([32, 38], f32, name="pk")
            c01 = cp.tile([32, 38], f32, name="c01")
            c23 = cp.tile([32, 38], f32, name="c23")
            nc.vector.tensor_add(c01[:, 0:32], hs[0:32, 0:32], tmpc[:, 0:32])
            nc.vector.tensor_add(c23[:, 0:32], tmpc[:, 32:64], tmpc[:, 64:96])
            nc.vector.tensor_add(c01[:, 32:38], hs[0:32, 128:134], tmpf[:, 0:6])
            nc.vector.tensor_add(c23[:, 32:38], tmpf[:, 6:12], tmpf[:, 12:18])
            nc.vector.tensor_add(pk, c01, c23)

            # ================= AllGather partials, reduce =================
            nc.sync.dma_start(ag_in.ap(), pk)
            nc.gpsimd.collective_compute(
                "AllGather", ALU.bypass,
                ins=[ag_in.ap().opt()], outs=[ag_out.ap().opt()],
                replica_groups=[list(range(NCORES))])
            g8 = cp.tile([32, 8 * 38], f32, name="g8")
            nc.sync.dma_start(
                g8.rearrange("p (i u) -> p i u", i=8),
                ag_out.ap().rearrange("(i d) u -> d i u", d=32))
            a4 = cp.tile([32, 152], f32, name="a4")
            nc.vector.tensor_add(a4, g8[:, 0:152], g8[:, 152:304])
            a2 = cp.tile([32, 76], f32, name="a2")
            nc.vector.tensor_add(a2, a4[:, 0:76], a4[:, 76:152])
            cf = cp.tile([32, 38], f32, name="cf")
            nc.vector.tensor_add(cf, a2[:, 0:38], a2[:, 38:76])

            # cnt, 1/cnt, M1T = (C/cnt)^T, FnT = (F/cnt)^T
            cnt = cp.tile([32, 1], f32, name="cnt")
            nc.vector.reduce_sum(cnt, cf[:, 0:32], axis=mybir.AxisListType.X)
            nc.vector.tensor_scalar(cnt, cnt, 1.0, None, ALU.max)
            inv = cp.tile([32, 1], f32, name="inv")
            nc.vector.reciprocal(inv, cnt)
            m1 = cp.tile([32, 32], f32, name="m1")
            nc.vector.tensor_scalar(m1, cf[:, 0:32], inv, None, ALU.mult)
            m1T = cp.tile([32, 32], f32, name="m1T")
            nc.vector.transpose(m1T, m1)
            fn_pad = cp.tile([32, 32], f32, name="fn_pad")
            nc.vector.memset(fn_pad, 0.0)
            nc.vector.tensor_scalar(
                fn_pad[:, 0:6], cf[:, 32:38], inv, None, ALU.mult)
            fnT = cp.tile([32, 32], f32, name="fnT")
            nc.vector.transpose(fnT, fn_pad)

            # ================= h0 = ne_w[nt] + te_w[tr] ===================
            nt_c16 = cp.tile([32, 1], i16, name="nt_c16")
            tr_c16 = cp.tile([32, 1], i16, name="tr_c16")
            nc.sync.dma_start(nt_c16, nt_d[:, 0:1])
            nc.sync.dma_start(tr_c16, tr_d[:, 0:1])
            nt_col = cp.tile([32, 1], f32, name="nt_col")
            tr_col = cp.tile([32, 1], f32, name="tr_col")
            nc.vector.tensor_copy(nt_col, nt_c16)
            nc.vector.tensor_copy(tr_col, tr_c16)
            # NT[node, t] = (nt[node] == t) then transpose to [t, node]
            nt_oh = cp.tile([32, 32], f32, name="nt_oh")
            tr_oh = cp.tile([32, 32], f32, name="tr_oh")
            nc.vector.tensor_scalar(nt_oh, iota_mat, nt_col, None,
                                    ALU.is_equal)
            nc.vector.tensor_scalar(tr_oh, iota_mat, tr_col, None,
                                    ALU.is_equal)
            ntT = cp.tile([32, 32], f32, name="ntT")
            trT = cp.tile([32, 32], f32, name="trT")
            nc.vector.transpose(ntT, nt_oh)
            nc.vector.transpose(trT, tr_oh)
            h0_ps = pp.tile([32, 128], f32, name="h0_ps", tag="psB")
            nc.tensor.matmul(h0_ps, ntT[0:20, :], ne_w, start=True, stop=False)
            nc.tensor.matmul(h0_ps, trT[0:6, :], te_w, start=False, stop=True)
            h_sb = vp.tile([32, 128], f32, name="h_sb", tag="h_sb")
            nc.vector.tensor_copy(h_sb, h0_ps)
            hT_ps0 = pp.tile([128, 32], f32, name="hT_ps0", tag="psE")
            nc.tensor.transpose(hT_ps0, h_sb, ident[0:32, 0:32])
            hT_sb = vp.tile([128, 32], f32, name="hT_sb", tag="hT_sb")
            nc.vector.tensor_copy(hT_sb, hT_ps0)

            # ================= 5 GRU iterations ===========================
            for it in range(5):
                aggT_ps = pp.tile([128, 32], f32, name="aggT_ps", tag="psA")
                nc.tensor.matmul(aggT_ps, h_sb, m1T, start=True, stop=False)
                nc.tensor.matmul(aggT_ps, ef_w, fnT[0:6, :],
                                 start=False, stop=True)
                aggT = vp.tile([128, 32], f32, name="aggT", tag="aggT")
                nc.vector.tensor_copy(aggT, aggT_ps)

                g_rz = pp.tile([32, 256], f32, name="g_rz", tag="psB")
                nc.tensor.matmul(g_rz, aggT, w_ihT[:, 0:256],
                                 start=True, stop=False)
                nc.tensor.matmul(g_rz, hT_sb, w_hhT[:, 0:256],
                                 start=False, stop=False)
                nc.tensor.matmul(g_rz, ones_row[0:1, 0:32], bsum[:, 0:256],
                                 start=False, stop=True)
                hn_ps = pp.tile([32, 128], f32, name="hn_ps", tag="psC")
                nc.tensor.matmul(hn_ps, hT_sb, w_hhT[:, 256:384],
                                 start=True, stop=False)
                nc.tensor.matmul(hn_ps, ones_row[0:1, 0:32], b_hh[:, 256:384],
                                 start=False, stop=True)
                in_ps = pp.tile([32, 128], f32, name="in_ps", tag="psD")
                nc.tensor.matmul(in_ps, aggT, w_ihT[:, 256:384],
                                 start=True, stop=False)
                nc.tensor.matmul(in_ps, ones_row[0:1, 0:32], b_ih[:, 256:384],
                                 start=False, stop=True)

                rz = vp.tile([32, 256], f32, name="rz", tag="rz")
                nc.scalar.activation(rz, g_rz, AF.Sigmoid)
                t1 = vp.tile([32, 128], f32, name="t1", tag="t1")
                nc.vector.tensor_tensor(t1, rz[:, 0:128], hn_ps, ALU.mult)
                t2 = vp.tile([32, 128], f32, name="t2", tag="t2")
                nc.vector.tensor_tensor(t2, t1, in_ps, ALU.add)
                n_sb = vp.tile([32, 128], f32, name="n_sb", tag="n_sb")
                nc.scalar.activation(n_sb, t2, AF.Tanh)

                d1 = vp.tile([32, 128], f32, name="d1", tag="d1")
                nc.vector.tensor_sub(d1, h_sb, n_sb)
                t3 = vp.tile([32, 128], f32, name="t3", tag="t3")
                nc.vector.tensor_tensor(t3, rz[:, 128:256], d1, ALU.mult)
                x_sb = vp.tile([32, 128], f32, name="x_sb", tag="x_sb")
                sx = vp.tile([32, 1], f32, name="sx", tag="sx")
                nc.vector.tensor_add(x_sb, t3, n_sb)
                nc.vector.reduce_sum(sx, x_sb, axis=mybir.AxisListType.X)
                xsq = vp.tile([32, 128], f32, name="xsq", tag="xsq")
                sxx = vp.tile([32, 1], f32, name="sxx", tag="sxx")
                nc.scalar.activation(xsq, x_sb, AF.Square, accum_out=sxx)

                mvec = vp.tile([32, 1], f32, name="mvec", tag="mvec")
                nc.vector.tensor_scalar(mvec, sx, 1.0 / 128, None, ALU.mult)
                av = vp.tile([32, 1], f32, name="av", tag="av")
                nc.vector.tensor_scalar(av, sxx, 1.0 / 128, EPS,
                                        ALU.mult, ALU.add)
                bv = vp.tile([32, 1], f32, name="bv", tag="bv")
                nc.vector.tensor_scalar(bv, mvec, mvec, None, ALU.mult)
                uv = vp.tile([32, 1], f32, name="uv", tag="uv")
                nc.vector.tensor_sub(uv, av, bv)
                isg, _ = _sqrt_newton(nc, vp, uv, "it")

                xn = vp.tile([32, 128], f32, name="xn", tag="xn")
                nc.vector.tensor_scalar(xn, x_sb, mvec, isg,
                                        ALU.subtract, ALU.mult)
                xnT_ps = pp.tile([128, 32], f32, name="xnT_ps", tag="psE")
                nc.tensor.transpose(xnT_ps, xn, ident[0:32, 0:32])
                hT_sb = vp.tile([128, 32], f32, name="hT_sb", tag="hT_sb")
                nc.scalar.activation(hT_sb, xnT_ps, AF.Identity,
                                     bias=ln_b, scale=ln_g)
                hN_ps = pp.tile([32, 128], f32, name="hN_ps", tag="psF")
                nc.tensor.transpose(hN_ps, hT_sb, ident)
                h_sb = vp.tile([32, 128], f32, name="h_sb", tag="h_sb")
                nc.vector.tensor_copy(h_sb, hN_ps)

            # ================= head: pool + fc1 + LN2 + relu + fc2 ========
            mean_ps = pp.tile([128, 1], f32, name="mean_ps", tag="psE")
            nc.tensor.matmul(mean_ps, h_sb, ones_col[0:32, 0:1],
                             start=True, stop=True)
            mean_sb = cp.tile([128, 1], f32, name="mean_sb")
            nc.scalar.activation(mean_sb, mean_ps, AF.Identity, scale=1.0 / 32)
            max_sb = cp.tile([128, 1], f32, name="max_sb")
            nc.vector.reduce_max(max_sb, hT_sb, axis=mybir.AxisListType.X)

            x1_ps = pp.tile([128, 1], f32, name="x1_ps", tag="psF")
            nc.tensor.matmul(x1_ps, fc1T_a, mean_sb, start=True, stop=False)
            nc.tensor.matmul(x1_ps, fc1T_b, max_sb, start=False, stop=True)
            st_in = cp.tile([128, 2], f32, name="st_in")
            nc.vector.tensor_add(st_in[:, 0:1], x1_ps, fc1_b)
            nc.scalar.activation(st_in[:, 1:2], st_in[:, 0:1], AF.Square)
            st_ps = pp.tile([1, 2], f32, name="st_ps", tag="psC")
            nc.tensor.matmul(st_ps, ones_col, st_in, start=True, stop=True)

            m2 = cp.tile([1, 1], f32, name="m2")
            nc.vector.tensor_scalar(m2, st_ps[0:1, 0:1], 1.0 / 128, None,
                                    ALU.mult)
            a2v = cp.tile([1, 1], f32, name="a2v")
            nc.vector.tensor_scalar(a2v, st_ps[0:1, 1:2], 1.0 / 128, EPS,
                                    ALU.mult, ALU.add)
            b2v = cp.tile([1, 1], f32, name="b2v")
            nc.vector.tensor_scalar(b2v, m2, m2, None, ALU.mult)
            u2 = cp.tile([1, 1], f32, name="u2")
            nc.vector.tensor_sub(u2, a2v, b2v)
            isg2, _ = _sqrt_newton(nc, cp, u2, "hd")

            # broadcast m2, isg2 across partitions via rank-1 PE matmul
            mi2 = cp.tile([1, 2], f32, name="mi2")
            nc.vector.tensor_copy(mi2[:, 0:1], m2)
            nc.vector.tensor_copy(mi2[:, 1:2], isg2)
            mi2b_ps = pp.tile([128, 2], f32, name="mi2b_ps", tag="psE")
            nc.tensor.matmul(mi2b_ps, ones_row, mi2, start=True, stop=True)
            mi2b = cp.tile([128, 2], f32, name="mi2b")
            nc.vector.tensor_copy(mi2b, mi2b_ps)
            xn2 = cp.tile([128, 1], f32, name="xn2")
            nc.vector.tensor_scalar(xn2, st_in[:, 0:1], mi2b[:, 0:1],
                                    mi2b[:, 1:2], ALU.subtract, ALU.mult)
            relu2 = cp.tile([128, 1], f32, name="relu2")
            nc.scalar.activation(relu2, xn2, AF.Relu, bias=ln2_b, scale=ln2_g)

            out_ps = pp.tile([1, 1], f32, name="out_ps", tag="psD")
            nc.tensor.matmul(out_ps, relu2, fc2_col, start=True, stop=True)
            out_sb = cp.tile([1, 1], f32, name="out_sb")
            nc.vector.tensor_add(out_sb, out_ps, fc2_b)
            nc.sync.dma_start(out_d.ap(), out_sb)

    _split_excess_waits(nc)
    return nc


_PROGRAM = None


def _get_program():
    global _PROGRAM
    if _PROGRAM is None:
        _PROGRAM = build_program()
    return _PROGRAM


def make_in_maps(inputs):
    """Shard FULL inputs into per-core in_maps (host-side: views/pads only)."""
    def pad_shard(a):
        a = np.asarray(a, dtype=np.int64)
        p = np.full(E_PAD, 32, dtype=np.int64)
        p[:E_FULL] = a
        return [np.ascontiguousarray(p[c * EPC:(c + 1) * EPC])
                .view(np.int16).reshape(128, 4 * W) for c in range(NCORES)]

    es_s = pad_shard(inputs["es"])
    ed_s = pad_shard(inputs["ed"])
    ef_s = pad_shard(inputs["ef"])

    def f(x, shape):
        return np.ascontiguousarray(
            np.asarray(x, dtype=np.float32).reshape(shape))

    common = {
        "nt": np.ascontiguousarray(np.asarray(inputs["nt"], np.int64))
        .view(np.int16).reshape(32, 4),
        "tr": np.ascontiguousarray(np.asarray(inputs["tr"], np.int64))
        .view(np.int16).reshape(32, 4),
        "ne_w": f(inputs["ne_w"], (20, DIM)),
        "te_w": f(inputs["te_w"], (6, DIM)),
        "ef_w": f(inputs["ef_w"], (6, DIM)),
        "w_ih": f(inputs["w_ih"], (384, DIM)),
        "w_hh": f(inputs["w_hh"], (384, DIM)),
        "b_ih": f(inputs["b_ih"], (1, 384)),
        "b_hh": f(inputs["b_hh"], (1, 384)),
        "ln_g": f(inputs["ln_g"], (DIM, 1)),
        "ln_b": f(inputs["ln_b"], (DIM, 1)),
        "fc1_w": f(inputs["fc1_w"], (DIM, 2 * DIM)),
        "fc1_b": f(inputs["fc1_b"], (DIM, 1)),
        "ln2_g": f(inputs["ln2_g"], (DIM, 1)),
        "ln2_b": f(inputs["ln2_b"], (DIM, 1)),
        "fc2_w": f(inputs["fc2_w"], (1, DIM)),
        "fc2_b": f(inputs["fc2_b"], (1, 1)),
        "ident128": np.eye(128, dtype=np.float32),
        "ones_row": np.ones((1, 128), np.float32),
        "ones_col": np.ones((128, 1), np.float32),
        "iota_col": np.arange(128, dtype=np.float32).reshape(128, 1),
        "iota_mat": np.broadcast_to(
            np.arange(32, dtype=np.float32), (32, 32)).copy(),
        "iota_row_bf": np.broadcast_to(
            np.arange(32, dtype=np.float32).astype(ml_dtypes.bfloat16),
            (128, 32)).copy(),
    }
    in_maps = []
    for c in range(NCORES):
        m = dict(common)
        m["es"] = es_s[c]
        m["ed"] = ed_s[c]
        m["ef"] = ef_s[c]
        in_maps.append(m)
    return in_maps


def kernel(**inputs) -> np.ndarray:
    nc = _get_program()
    in_maps = make_in_maps(inputs)
    res = run_bass_kernel_spmd(nc, in_maps, core_ids=list(range(NCORES)))
    return np.asarray(res.results[0]["out"], np.float32).reshape(())



# revision 12
# speedup vs baseline: 1.3707x; 1.3707x over previous
"""Trainium2 Bass kernel for nn_Detector (GNN message passing).

Math: the reference's per-iteration edge aggregation
    agg = segment_sum((h[src] + ef_w[ef]) * valid, by=ed)[:N] / cnt
is linear in h and ef_w, so it factors through two tiny count histograms
built in ONE pass over the edge index arrays:
    C[d, s] = #valid edges s->d          (32x32)
    F[d, t] = #valid edges into d with feature t   (32x6)
    agg = (C @ h + F @ ef_w) / cnt,   cnt = max(rowsum(C), 1)

Histogram on device: per 128-edge chunk, C_chunk = D^T S via the PE with
one-hot rows; 4 chunks are packed per matmul (block-diagonal trick).  The
source one-hots are DIGIT-PACKED base 8192: column j carries
[es==2j] + 8192*[es==2j+1], so the moving operand is 22 wide (16 packed S
+ 6 plain F) instead of 38, and PSUM accumulates both source counts
exactly in fp32 (per-pair global count ~455 << 8192; 8192*455 << 2^24).
One-hots are built value-major with unit-stride operands so the DVE runs
in its 4x perf mode (broadcasts only on outer dims).

Precision: all matmul inputs are bf16 (fp32 PSUM); all elementwise state
stays fp32 (bf16 state would cost ~1e-2 relative error at the head's
cancellation-heavy fc2 dot; bf16-at-matmul-only lands ~2.4e-3).

Distribution: edges sharded across 8 cores (int16 index shards), partial
histograms [32,22] AllReduced, then every core runs the identical
5-iteration GRU + head; core 0's scalar is returned.
"""

import ml_dtypes
import numpy as np

import concourse.bass as bass
import concourse.mybir as mybir
import concourse.tile as tile
from concourse.bass_utils import run_bass_kernel_spmd

dt = mybir.dt
AF = mybir.ActivationFunctionType
ALU = mybir.AluOpType

NCORES = 8
E_FULL = 400000
W = 400                    # edge columns per partition row
EPC = 128 * W              # 51200 padded edges per core
E_PAD = NCORES * EPC
SGW = 100                  # supergroup width (chunk columns)
NSG = W // SGW             # 4 supergroups
NGRP = SGW // 4            # 25 matmul groups (4 chunks) per supergroup
DIM = 128
N = 32
EPS = 1e-5
BASE = 8192.0              # digit-packing base for source one-hots
RSQRT_MAGIC = 0x5F3759DF
MAX_WAITS = 1

f32 = dt.float32
bf16 = dt.bfloat16
i16 = dt.int16
i32 = dt.int32

# wbf (bf16 [128, 2112]) column layout
O_WIH = 0          # w_ihT [128, 384]
O_WHH_RZ = 384     # [128, 256]
O_WHH_N = 640      # [128, 128]
O_FC1A = 768       # [128, 128]
O_FC1B = 896       # [128, 128]
O_ID128 = 1024     # identity [128, 128]
O_NEW = 1152       # ne_w rows 0:20 [*, 128]
O_TEW = 1280       # te_w rows 0:6
O_EFW = 1408       # ef_w rows 0:6
O_BIAS = 1536      # bias_cat row 0 [1, 512]
O_ONEC = 2048      # ones col [128, 1]
O_ONER32 = 2049    # ones row [1, 32] (row 0)
WBF_COLS = 2112

# wf32 (f32 [128, 552]) column layout
F_LN2G = 2
F_LN2B = 3
F_FC1B = 4
F_FC2 = 5          # fc2 col [128, 1]
F_ONEC = 6         # ones col f32
F_IOTA = 7         # iota_mat rows 0:32, cols 7:39
F_FC2B = 39        # fc2_b at [0,39]
F_ONER = 40        # ones row [1, 128] at row 0
F_GROW = 168       # ln_g as a row [1, 128] at row 0
F_BROW = 296       # ln_b as a row [1, 128] at row 0
F_SEL = 424        # 4 selection matrices [128, 32] each: Sel_c[4v+c, v] = 1
WF32_COLS = 552


def _split_excess_waits(nc):
    """Split instructions carrying more than MAX_WAITS sync-wait conditions
    into preceding same-engine NOPs (walrus codegen limit)."""
    for blk in nc.main_func.blocks:
        insts = blk.instructions
        i = 0
        while i < len(insts):
            inst = insts[i]
            si = inst.sync_info
            if si is not None and len(si.on_wait) > MAX_WAITS:
                waits = list(si.on_wait)
                keep = waits[-MAX_WAITS:]
                rest = waits[:-MAX_WAITS]
                new_nops = []
                while rest:
                    chunk, rest = rest[:MAX_WAITS], rest[MAX_WAITS:]
                    nop = mybir.InstNoOp(
                        name=f"waitsplit-{nc.next_id()}", ins=[], outs=[])
                    nop.engine = inst.engine
                    nop.sync_info = mybir.SyncInfo(on_wait=chunk, on_update=[])
                    nc.register_instruction(nop, overwrite=True)
                    new_nops.append(nop)
                inst.sync_info = mybir.SyncInfo(
                    on_wait=keep, on_update=list(si.on_update))
                for j, nop in enumerate(new_nops):
                    insts.insert(i + j, nop)
                i += len(new_nops)
            i += 1


def _sqrt_newton(nc, vp, u, tag_prefix):
    """1/sqrt(u) for u [P,1] fp32 via rsqrt bit-hack + 2 Newton iterations
    (ACT Sqrt/Rsqrt are banned/table-expensive)."""
    P = u.shape[0]
    y = vp.tile([P, 1], f32, name=f"{tag_prefix}_y", tag=f"{tag_prefix}_y")
    a = vp.tile([P, 1], f32, name=f"{tag_prefix}_a", tag=f"{tag_prefix}_a")
    # y0 bits = MAGIC - (u_bits >> 1), via c - x = (~x) + (c + 1)
    # (bitwise and arith ALU ops cannot share one instruction)
    nc.vector.tensor_scalar(
        y.bitcast(i32), u.bitcast(i32), 1, None, ALU.logical_shift_right)
    nc.vector.tensor_scalar(
        y.bitcast(i32), y.bitcast(i32), -1, None, ALU.bitwise_xor)
    nc.vector.tensor_scalar(
        y.bitcast(i32), y.bitcast(i32), RSQRT_MAGIC + 1, None, ALU.add)
    for _ in range(2):
        nc.vector.tensor_mul(a, y, y)
        nc.vector.tensor_mul(a, a, u)
        nc.vector.tensor_scalar(a, a, -0.5, 1.5, ALU.mult, ALU.add)
        nc.vector.tensor_mul(y, y, a)
    return y


def build_program():
    # this walrus snapshot cannot encode the Pool RANGE_CLEAR InstISA that
    # TileContext's exit emits via clear_and_free_semaphores; skip the
    # sem-clear ISA (keep dma_reset + bookkeeping).
    _orig_clear = bass.Bass.clear_and_free_semaphores

    def _clear_no_isa(self, sems):
        if not sems:
            return
        sem_nums = [
            s.num if isinstance(s, bass.SemaphoreHandle) else s for s in sems
        ]
        from concourse.bass import compact_to_ranges
        for sem_range in compact_to_ranges(sem_nums):
            self.gpsimd.dma_reset(sem_range)
        self._state.prepend_free_semaphores(sem_nums)
        for poison_set in self._tile_sem_poison_stack:
            poison_set.update(sem_nums)

    bass.Bass.clear_and_free_semaphores = _clear_no_isa
    try:
        return _build_program_inner()
    finally:
        bass.Bass.clear_and_free_semaphores = _orig_clear


def _build_program_inner():
    nc = bass.Bass(trn_type="TRN2")

    # ---- DRAM I/O ---------------------------------------------------------
    es_d = nc.dram_tensor("es", [128, W], i16, kind="ExternalInput")
    ed_d = nc.dram_tensor("ed", [128, W], i16, kind="ExternalInput")
    ef_d = nc.dram_tensor("ef", [128, W], i16, kind="ExternalInput")
    iota_d = nc.dram_tensor("iota16", [128, 128], i16, kind="ExternalInput")
    wbf_d = nc.dram_tensor("wbf", [128, WBF_COLS], bf16, kind="ExternalInput")
    wf32_d = nc.dram_tensor("wf32", [128, WF32_COLS], f32, kind="ExternalInput")
    nttr_d = nc.dram_tensor("nttr", [32, 2], i16, kind="ExternalInput")
    out_d = nc.dram_tensor("out", [1, 1], f32, kind="ExternalOutput")

    ag_in = nc.dram_tensor("ag_in", [32, 22], f32)
    ag_out = nc.dram_tensor("ag_out", [32, 22], f32, addr_space="Shared")

    with tile.TileContext(nc) as tc:
        with (
            tc.tile_pool(name="cst", bufs=1) as cp,
            tc.tile_pool(name="var", bufs=2) as vp,
            tc.tile_pool(name="ps", bufs=1, space="PSUM") as pp,
        ):
            # ================= input DMAs =================================
            wbf = cp.tile([128, WBF_COLS], bf16, name="wbf")
            nc.sync.dma_start(wbf, wbf_d[:, :])
            wf32 = cp.tile([128, WF32_COLS], f32, name="wf32")
            nc.sync.dma_start(wf32, wf32_d[:, :])
            iota16 = cp.tile([128, 128], i16, name="iota16")
            nc.sync.dma_start(iota16, iota_d[:, :])
            es = cp.tile([128, W], i16, name="es")
            nc.sync.dma_start(es, es_d[:, :])
            ed = cp.tile([128, W], i16, name="ed")
            nc.sync.dma_start(ed, ed_d[:, :])
            ef = cp.tile([128, W], i16, name="ef")
            nc.sync.dma_start(ef, ef_d[:, :])
            nttr = cp.tile([32, 2], i16, name="nttr")
            nc.sync.dma_start(nttr, nttr_d[:, :])

            ident128 = wbf[:, O_ID128:O_ID128 + 128]
            ident32 = wbf[0:32, O_ID128:O_ID128 + 32]
            ones_col_bf = wbf[0:32, O_ONEC:O_ONEC + 1]
            ones_row32 = wbf[0:1, O_ONER32:O_ONER32 + 32]

            # ================= edge phase =================================
            # preps: halved source values + parity weight w = 1 or BASE
            es1 = cp.tile([128, W], i16, name="es1")
            nc.vector.tensor_scalar(es1, es, 1, None, ALU.logical_shift_right)
            par_s = cp.tile([128, W], i16, name="par_s")
            nc.vector.tensor_scalar(par_s, es, 1, None, ALU.bitwise_and)
            ws32 = cp.tile([128, W], f32, name="ws32")
            nc.vector.tensor_scalar(ws32, par_s, BASE - 1.0, 1.0,
                                    ALU.mult, ALU.add)
            ws_bf = cp.tile([128, W], bf16, name="ws_bf")
            nc.vector.tensor_copy(ws_bf, ws32)

            # iota_gm [128, (v32, c4)]: element (v, c) = v
            iota_vc = iota16.rearrange("p (v c) -> p v c", v=32)

            # one-hots in GROUP-major layout so each matmul group's
            # stationary is a plain contiguous 128-column slice (the BIR
            # verifier requires a single free dim on the weights AP):
            #   dsg [128, (G, v32, c4)]   sfg [128, (G, j22, c4)]
            hist = pp.tile([128, 88], f32, name="hist", tag="psA")
            for sg in range(NSG):
                sl = slice(sg * SGW, (sg + 1) * SGW)
                ed_g = ed[:, sl].rearrange("p (G c) -> p G c", c=4)
                es1_g = es1[:, sl].rearrange("p (G c) -> p G c", c=4)
                ef_g = ef[:, sl].rearrange("p (G c) -> p G c", c=4)
                ws_g = ws_bf[:, sl].rearrange("p (G c) -> p G c", c=4)

                dsg = vp.tile([128, 32 * SGW], bf16, name="dsg", tag="dsg")
                dsg_r = dsg.rearrange("p (G v c) -> p G v c", v=32, c=4)
                nc.vector.tensor_tensor(
                    dsg_r,
                    ed_g.unsqueeze(2).broadcast_to([128, NGRP, 32, 4]),
                    iota_vc.unsqueeze(1).broadcast_to([128, NGRP, 32, 4]),
                    ALU.is_equal)
                sfg = vp.tile([128, 22 * SGW], bf16, name="sfg", tag="sfg")
                sfg_r = sfg.rearrange("p (G j c) -> p G j c", j=22, c=4)
                oh16 = vp.tile([128, 16 * SGW], bf16, name="oh16", tag="oh16")
                oh16_r = oh16.rearrange("p (G j c) -> p G j c", j=16, c=4)
                nc.vector.tensor_tensor(
                    oh16_r,
                    es1_g.unsqueeze(2).broadcast_to([128, NGRP, 16, 4]),
                    iota_vc[:, 0:16, :].unsqueeze(1)
                    .broadcast_to([128, NGRP, 16, 4]),
                    ALU.is_equal)
                nc.vector.tensor_tensor(
                    sfg_r[:, :, 0:16, :], oh16_r,
                    ws_g.unsqueeze(2).broadcast_to([128, NGRP, 16, 4]),
                    ALU.mult)
                nc.vector.tensor_tensor(
                    sfg_r[:, :, 16:22, :],
                    ef_g.unsqueeze(2).broadcast_to([128, NGRP, 6, 4]),
                    iota_vc[:, 0:6, :].unsqueeze(1)
                    .broadcast_to([128, NGRP, 6, 4]),
                    ALU.is_equal)

                # moving operand iterates (c outer, j inner): out col = 22c+j
                sfg_m = sfg.rearrange("p (G j c) -> p G c j", j=22, c=4)
                for g in range(NGRP):
                    lhsT = dsg[:, 128 * g:128 * (g + 1)]
                    rhs = sfg_m[:, g]
                    nc.tensor.matmul(
                        hist, lhsT, rhs,
                        start=(sg == 0 and g == 0),
                        stop=(sg == NSG - 1 and g == NGRP - 1))

            # sum the 4 diagonal [32,22] blocks: PSUM partition index is
            # 4v+c, col block c is [22c:22c+22]; selection matmuls pick
            # Sel_c[4v+c, v]=1 (fp32, exact for integer counts)
            hs = cp.tile([128, 88], f32, name="hs")
            nc.scalar.copy(hs, hist)
            pk_ps = pp.tile([32, 22], f32, name="pk_ps", tag="psB")
            for c in range(4):
                nc.tensor.matmul(
                    pk_ps, wf32[:, F_SEL + 32 * c:F_SEL + 32 * (c + 1)],
                    hs[:, 22 * c:22 * (c + 1)],
                    start=(c == 0), stop=(c == 3))
            pk = cp.tile([32, 22], f32, name="pk")
            nc.vector.tensor_copy(pk, pk_ps)

            # ================= AllReduce partials =========================
            nc.sync.dma_start(ag_in.ap(), pk)
            nc.gpsimd.collective_compute(
                "AllReduce", ALU.add,
                ins=[ag_in.ap().opt()], outs=[ag_out.ap().opt()],
                replica_groups=[list(range(NCORES))])

            # ====== h0 / ln-row prep (independent; overlaps collective) ===
            nttr_f = cp.tile([32, 2], f32, name="nttr_f")
            nc.vector.tensor_copy(nttr_f, nttr)
            iota_mat = wf32[0:32, F_IOTA:F_IOTA + 32]
            nt_oh = cp.tile([32, 32], bf16, name="nt_oh")
            tr_oh = cp.tile([32, 32], bf16, name="tr_oh")
            nc.vector.tensor_scalar(nt_oh, iota_mat, nttr_f[:, 0:1], None,
                                    ALU.is_equal)
            nc.vector.tensor_scalar(tr_oh, iota_mat, nttr_f[:, 1:2], None,
                                    ALU.is_equal)
            ntT = cp.tile([32, 32], bf16, name="ntT")
            trT = cp.tile([32, 32], bf16, name="trT")
            nc.vector.transpose(ntT, nt_oh)
            nc.vector.transpose(trT, tr_oh)
            h0_ps = pp.tile([32, 128], f32, name="h0_ps", tag="psB")
            nc.tensor.matmul(h0_ps, ntT[0:20, :], wbf[0:20, O_NEW:O_NEW + 128],
                             start=True, stop=False)
            nc.tensor.matmul(h0_ps, trT[0:6, :], wbf[0:6, O_TEW:O_TEW + 128],
                             start=False, stop=True)
            h_f = vp.tile([32, 128], f32, name="h_f", tag="h_f")
            nc.vector.tensor_copy(h_f, h0_ps)
            h_bf = vp.tile([32, 128], bf16, name="h_bf", tag="h_bf")
            nc.vector.tensor_copy(h_bf, h0_ps)
            hT_ps0 = pp.tile([128, 32], bf16, name="hT_ps0", tag="psC")
            nc.tensor.transpose(hT_ps0, h_bf, ident32)
            hT_bf = vp.tile([128, 32], bf16, name="hT_bf", tag="hT_bf")
            nc.scalar.copy(hT_bf, hT_ps0)

            # ln_g/ln_b broadcast to full [32,128] tiles (free-dim affine)
            gb_ps = pp.tile([32, 256], f32, name="gb_ps", tag="psD")
            nc.tensor.matmul(gb_ps, wf32[0:1, F_ONER:F_ONER + 32],
                             wf32[0:1, F_GROW:F_GROW + 256],
                             start=True, stop=True)
            gb = cp.tile([32, 256], f32, name="gb")
            nc.vector.tensor_copy(gb, gb_ps)
            g_full = gb[:, 0:128]
            b_full = gb[:, 128:256]

            # ================= decode histogram, build M1/Fn/q ============
            cf = cp.tile([32, 22], f32, name="cf")
            nc.sync.dma_start(cf, ag_out.ap())
            todd = cp.tile([32, 16], f32, name="todd")
            nc.vector.tensor_scalar(todd, cf[:, 0:16], 1.0 / BASE, None,
                                    ALU.mult)
            todd_i = cp.tile([32, 16], i32, name="todd_i")
            nc.vector.tensor_copy(todd_i, todd)        # trunc/round -> c_odd
            codd = cp.tile([32, 16], f32, name="codd")
            nc.vector.tensor_copy(codd, todd_i)
            ceven = cp.tile([32, 16], f32, name="ceven")
            nc.vector.scalar_tensor_tensor(
                ceven, codd, -BASE, cf[:, 0:16], ALU.mult, ALU.add)

            cm = cp.tile([32, 32], f32, name="cm")
            cm_v = cm.rearrange("p (s two) -> p s two", two=2)
            nc.vector.tensor_copy(cm_v[:, :, 0:1], ceven.unsqueeze(2))
            nc.vector.tensor_copy(cm_v[:, :, 1:2], codd.unsqueeze(2))
            cnt = cp.tile([32, 1], f32, name="cnt")
            nc.vector.reduce_sum(cnt, cm, axis=mybir.AxisListType.X)
            nc.vector.tensor_scalar(cnt, cnt, 1.0, None, ALU.max)
            inv = cp.tile([32, 1], f32, name="inv")
            nc.vector.reciprocal(inv, cnt)
            m1 = cp.tile([32, 32], f32, name="m1")
            nc.vector.tensor_scalar(m1, cm, inv, None, ALU.mult)
            m1t = cp.tile([32, 32], f32, name="m1t")
            nc.vector.transpose(m1t, m1)
            m1t_bf = cp.tile([32, 32], bf16, name="m1t_bf")
            nc.vector.tensor_copy(m1t_bf, m1t)

            fn_pad = cp.tile([32, 32], f32, name="fn_pad")
            nc.vector.memset(fn_pad, 0.0)
            nc.vector.tensor_scalar(fn_pad[:, 0:6], cf[:, 16:22], inv, None,
                                    ALU.mult)
            fnt = cp.tile([32, 32], f32, name="fnt")
            nc.vector.transpose(fnt, fn_pad)
            fnt_bf = cp.tile([32, 32], bf16, name="fnt_bf")
            nc.vector.tensor_copy(fnt_bf, fnt)

            q_ps = pp.tile([32, 128], f32, name="q_ps", tag="psD")
            nc.tensor.matmul(q_ps, fnt_bf[0:6, :], wbf[0:6, O_EFW:O_EFW + 128],
                             start=True, stop=True)
            q_bf = cp.tile([32, 128], bf16, name="q_bf")
            nc.scalar.copy(q_bf, q_ps)

            # ================= 5 GRU iterations ===========================
            for it in range(5):
                # gate PSUM [32, 512]: [rz_sum(256) | i_n(128) | h_n(128)]
                g_all = pp.tile([32, 512], f32, name="g_all", tag="psB")
                nc.tensor.matmul(g_all, ones_row32,
                                 wbf[0:1, O_BIAS:O_BIAS + 512],
                                 start=True, stop=False)
                nc.tensor.matmul(g_all[:, 0:256], hT_bf,
                                 wbf[:, O_WHH_RZ:O_WHH_RZ + 256],
                                 start=False, stop=False, skip_group_check=True)
                nc.tensor.matmul(g_all[:, 384:512], hT_bf,
                                 wbf[:, O_WHH_N:O_WHH_N + 128],
                                 start=False, stop=False, skip_group_check=True)

                aggT_ps = pp.tile([128, 32], f32, name="aggT_ps", tag="psC")
                nc.tensor.matmul(aggT_ps, h_bf, m1t_bf, start=True, stop=False)
                nc.tensor.matmul(aggT_ps, q_bf, ident32,
                                 start=False, stop=True)
                aggT = vp.tile([128, 32], bf16, name="aggT", tag="aggT")
                nc.scalar.copy(aggT, aggT_ps)

                nc.tensor.matmul(g_all[:, 0:384], aggT,
                                 wbf[:, O_WIH:O_WIH + 384],
                                 start=False, stop=True, skip_group_check=True)

                rz = vp.tile([32, 256], f32, name="rz", tag="rz")
                nc.scalar.activation(rz, g_all[:, 0:256], AF.Sigmoid)
                t1 = vp.tile([32, 128], f32, name="t1", tag="t1")
                nc.vector.tensor_tensor(t1, rz[:, 0:128], g_all[:, 384:512],
                                        ALU.mult)
                t2 = vp.tile([32, 128], f32, name="t2", tag="t2")
                nc.vector.tensor_tensor(t2, t1, g_all[:, 256:384], ALU.add)
                n_f = vp.tile([32, 128], f32, name="n_f", tag="n_f")
                nc.scalar.activation(n_f, t2, AF.Tanh)

                u = vp.tile([32, 128], f32, name="u", tag="u")
                nc.vector.tensor_sub(u, h_f, n_f)
                t3 = vp.tile([32, 128], f32, name="t3", tag="t3")
                nc.vector.tensor_tensor(t3, rz[:, 128:256], u, ALU.mult)
                hp = vp.tile([32, 128], f32, name="hp", tag="hp")
                nc.vector.tensor_add(hp, t3, n_f)

                st6 = vp.tile([32, 6], f32, name="st6", tag="st6")
                nc.vector.bn_stats(st6, hp)
                mv = vp.tile([32, 2], f32, name="mv", tag="mv")
                nc.vector.bn_aggr(mv, st6)
                uv = vp.tile([32, 1], f32, name="uv", tag="uv")
                nc.vector.tensor_scalar(uv, mv[:, 1:2], EPS, None, ALU.add)
                isg = _sqrt_newton(nc, vp, uv, "it")

                # h = ((hp - m) * g) * isg + b   (LN + affine, 2 fused ops)
                ta = vp.tile([32, 128], f32, name="ta", tag="ta")
                nc.vector.scalar_tensor_tensor(
                    ta, hp, mv[:, 0:1], g_full, ALU.subtract, ALU.mult)
                h_f = vp.tile([32, 128], f32, name="h_f", tag="h_f")
                nc.vector.scalar_tensor_tensor(
                    h_f, ta, isg, b_full, ALU.mult, ALU.add)

                h_bf = vp.tile([32, 128], bf16, name="h_bf", tag="h_bf")
                nc.vector.tensor_copy(h_bf, h_f)
                hT_ps = pp.tile([128, 32], bf16, name="hT_ps", tag="psE")
                nc.tensor.transpose(hT_ps, h_bf, ident32)
                hT_bf = vp.tile([128, 32], bf16, name="hT_bf", tag="hT_bf")
                nc.scalar.copy(hT_bf, hT_ps)

            # ================= head =======================================
            mean_ps = pp.tile([128, 1], f32, name="mean_ps", tag="psC")
            nc.tensor.matmul(mean_ps, h_bf, ones_col_bf, start=True, stop=True)
            mean_bf = cp.tile([128, 1], bf16, name="mean_bf")
            nc.scalar.activation(mean_bf, mean_ps, AF.Identity, scale=1.0 / 32)
            max_f = cp.tile([128, 1], f32, name="max_f")
            nc.vector.reduce_max(max_f, hT_bf, axis=mybir.AxisListType.X)
            max_bf = cp.tile([128, 1], bf16, name="max_bf")
            nc.vector.tensor_copy(max_bf, max_f)

            x1_ps = pp.tile([128, 1], f32, name="x1_ps", tag="psD")
            nc.tensor.matmul(x1_ps, wbf[:, O_FC1A:O_FC1A + 128], mean_bf,
                             start=True, stop=False)
            nc.tensor.matmul(x1_ps, wbf[:, O_FC1B:O_FC1B + 128], max_bf,
                             start=False, stop=True)
            st_in = cp.tile([128, 2], f32, name="st_in")
            nc.vector.tensor_add(st_in[:, 0:1], x1_ps,
                                 wf32[:, F_FC1B:F_FC1B + 1])
            nc.scalar.activation(st_in[:, 1:2], st_in[:, 0:1], AF.Square)
            st_ps = pp.tile([1, 2], f32, name="st_ps", tag="psE")
            nc.tensor.matmul(st_ps, wf32[:, F_ONEC:F_ONEC + 1], st_in,
                             start=True, stop=True)

            m2 = cp.tile([1, 1], f32, name="m2")
            nc.vector.tensor_scalar(m2, st_ps[0:1, 0:1], 1.0 / 128, None,
                                    ALU.mult)
            a2v = cp.tile([1, 1], f32, name="a2v")
            nc.vector.tensor_scalar(a2v, st_ps[0:1, 1:2], 1.0 / 128, EPS,
                                    ALU.mult, ALU.add)
            b2v = cp.tile([1, 1], f32, name="b2v")
            nc.vector.tensor_scalar(b2v, m2, m2, None, ALU.mult)
            u2 = cp.tile([1, 1], f32, name="u2")
            nc.vector.tensor_sub(u2, a2v, b2v)
            isg2 = _sqrt_newton(nc, cp, u2, "hd")

            mi2 = cp.tile([1, 2], f32, name="mi2")
            nc.vector.tensor_copy(mi2[:, 0:1], m2)
            nc.vector.tensor_copy(mi2[:, 1:2], isg2)
            mi2b_ps = pp.tile([128, 2], f32, name="mi2b_ps", tag="psC")
            nc.tensor.matmul(mi2b_ps, wf32[0:1, F_ONER:F_ONER + 128], mi2,
                             start=True, stop=True)
            mi2b = cp.tile([128, 2], f32, name="mi2b")
            nc.vector.tensor_copy(mi2b, mi2b_ps)
            xn2 = cp.tile([128, 1], f32, name="xn2")
            nc.vector.tensor_scalar(xn2, st_in[:, 0:1], mi2b[:, 0:1],
                                    mi2b[:, 1:2], ALU.subtract, ALU.mult)
            relu2 = cp.tile([128, 1], f32, name="relu2")
            nc.scalar.activation(relu2, xn2, AF.Relu,
                                 bias=wf32[:, F_LN2B:F_LN2B + 1],
                                 scale=wf32[:, F_LN2G:F_LN2G + 1])

            out_ps = pp.tile([1, 1], f32, name="out_ps", tag="psE")
            nc.tensor.matmul(out_ps, relu2, wf32[:, F_FC2:F_FC2 + 1],
                             start=True, stop=True)
            out_sb = cp.tile([1, 1], f32, name="out_sb")
            nc.vector.tensor_add(out_sb, out_ps, wf32[0:1, F_FC2B:F_FC2B + 1])
            nc.sync.dma_start(out_d.ap(), out_sb)

    _split_excess_waits(nc)
    return nc


_PROGRAM = None


def _get_program():
    global _PROGRAM
    if _PROGRAM is None:
        _PROGRAM = build_program()
    return _PROGRAM


def make_in_maps(inputs):
    """Shard FULL inputs into per-core in_maps (host-side prep)."""
    bf = ml_dtypes.bfloat16

    def pad_shard(a):
        a = np.asarray(a, dtype=np.int64)
        p = np.full(E_PAD, 32, dtype=np.int16)
        p[:E_FULL] = a.astype(np.int16)
        return [np.ascontiguousarray(p[c * EPC:(c + 1) * EPC]).reshape(128, W)
                for c in range(NCORES)]

    es_s = pad_shard(inputs["es"])
    ed_s = pad_shard(inputs["ed"])
    ef_s = pad_shard(inputs["ef"])

    f = lambda x: np.asarray(x, dtype=np.float32)

    wbf = np.zeros((128, WBF_COLS), dtype=bf)
    wbf[:, O_WIH:O_WIH + 384] = f(inputs["w_ih"]).T.astype(bf)
    w_hh_t = f(inputs["w_hh"]).T
    wbf[:, O_WHH_RZ:O_WHH_RZ + 256] = w_hh_t[:, 0:256].astype(bf)
    wbf[:, O_WHH_N:O_WHH_N + 128] = w_hh_t[:, 256:384].astype(bf)
    fc1 = f(inputs["fc1_w"])
    wbf[:, O_FC1A:O_FC1A + 128] = fc1[:, 0:128].T.astype(bf)
    wbf[:, O_FC1B:O_FC1B + 128] = fc1[:, 128:256].T.astype(bf)
    wbf[:, O_ID128:O_ID128 + 128] = np.eye(128, dtype=np.float32).astype(bf)
    wbf[0:20, O_NEW:O_NEW + 128] = f(inputs["ne_w"]).astype(bf)
    wbf[0:6, O_TEW:O_TEW + 128] = f(inputs["te_w"]).astype(bf)
    wbf[0:6, O_EFW:O_EFW + 128] = f(inputs["ef_w"]).astype(bf)
    b_ih = f(inputs["b_ih"]).reshape(384)
    b_hh = f(inputs["b_hh"]).reshape(384)
    bias_cat = np.concatenate([
        b_ih[0:256] + b_hh[0:256],      # rz sum
        b_ih[256:384],                  # i_n
        b_hh[256:384],                  # h_n
    ])
    wbf[0:1, O_BIAS:O_BIAS + 512] = bias_cat.reshape(1, 512).astype(bf)
    wbf[:, O_ONEC:O_ONEC + 1] = 1.0
    wbf[0:1, O_ONER32:O_ONER32 + 32] = 1.0

    wf32 = np.zeros((128, WF32_COLS), dtype=np.float32)
    wf32[:, F_LN2G] = f(inputs["ln2_g"]).reshape(128)
    wf32[:, F_LN2B] = f(inputs["ln2_b"]).reshape(128)
    wf32[:, F_FC1B] = f(inputs["fc1_b"]).reshape(128)
    wf32[:, F_FC2] = f(inputs["fc2_w"]).reshape(128)
    wf32[:, F_ONEC] = 1.0
    wf32[0:32, F_IOTA:F_IOTA + 32] = np.broadcast_to(
        np.arange(32, dtype=np.float32), (32, 32))
    wf32[0, F_FC2B] = float(np.asarray(inputs["fc2_b"]).reshape(()))
    wf32[0, F_ONER:F_ONER + 128] = 1.0
    wf32[0, F_GROW:F_GROW + 128] = f(inputs["ln_g"]).reshape(128)
    wf32[0, F_BROW:F_BROW + 128] = f(inputs["ln_b"]).reshape(128)
    for c in range(4):
        sel = np.zeros((128, 32), np.float32)
        sel[4 * np.arange(32) + c, np.arange(32)] = 1.0
        wf32[:, F_SEL + 32 * c:F_SEL + 32 * (c + 1)] = sel

    iota16 = np.ascontiguousarray(
        np.broadcast_to(
            np.repeat(np.arange(32, dtype=np.int16), 4).reshape(1, 128),
            (128, 128)))

    nttr = np.ascontiguousarray(np.stack([
        np.asarray(inputs["nt"], np.int64).astype(np.int16),
        np.asarray(inputs["tr"], np.int64).astype(np.int16),
    ], axis=1))

    common = {
        "wbf": wbf,
        "wf32": wf32,
        "iota16": iota16,
        "nttr": nttr,
    }
    in_maps = []
    for c in range(NCORES):
        m = dict(common)
        m["es"] = es_s[c]
        m["ed"] = ed_s[c]
        m["ef"] = ef_s[c]
        in_maps.append(m)
    return in_maps


def kernel(**inputs) -> np.ndarray:
    nc = _get_program()
    in_maps = make_in_maps(inputs)
    res = run_bass_kernel_spmd(nc, in_maps, core_ids=list(range(NCORES)))
    return np.asarray(res.results[0]["out"], np.float32).reshape(())
